# revision 18
# baseline (speedup 1.0000x reference)
"""Trainium2 Bass kernel for nn_EpsiLayer: per-channel causal full-length
time convolution  out[b,t,j] = P[b,t,j] + sum_{k<=t} g[k,j] * P[b,t-k,j].

Identity fold: with g'[0] = g[0] + 1, out = causal_conv(g', P) exactly.

Per channel j the conv is a lower-triangular Toeplitz (T x T) matmul.
Blocked into C=128 chunks: y_i += W_d @ x_{i-d},
W_d[p, a] = gpad[d*128 + a + p], gpad = 127 zeros ++ g' (bf16); the
moving operand is time-reversed within each block on the host so the
contraction pairs line up.  Each W_d is a 128x128 Hankel slice of the
dense sliding window wdense[p, e] = gpad[e + p].

The kernel is HBM-bound on the weight stream (the dense Toeplitz
expansion is ~124x redundant but must be materialized in SBUF for the
PE's stationary operand; fp8/int8 fail the 2e-2 error budget).  Two
mitigations:
  - bf16 output (halves output traffic; host casts back to fp32).
  - PE-side Hankel generation for the tail offsets d >= GEND0: the
    host ships a compact seed S8_d[p, y] = gpad[d*128 + p + 8*y]
    ([128,16] bf16 = 4 KB vs 32 KB dense).  On device, 8 shift-matrix
    matmuls (lhsT = sub-diagonal permutation Sigma_z, z=0..7) scatter
    the seed into PSUM columns z::8, giving H_d[p, 8y+z] =
    S8_d[p+z, y] for p < 121; the last 7 partitions (p+z >= 128 would
    need seed rows past 128) come from a tiny dense HBM strip.  A DVE
    copy casts PSUM -> SBUF bf16.  This moves ~1/4 of the weight
    stream off HBM into spare PE cycles.

Sharding: channel-parallel, NR=256 -> 32 channels per core, all B and T
per core, zero inter-core communication.
"""

import sys
import numpy as np

try:
    from concourse import bacc, tile  # noqa: F401
except ImportError:  # grading env may not have it on sys.path yet
    sys.path.insert(0, "/opt/trn_rl_repo")

import ml_dtypes

B, T, NR = 8, 4096, 256
C = 128
NB = T // C            # 32 time blocks
N_CORES = 8
CPC = NR // N_CORES    # 32 channels per core
COLS = CPC * NB * B    # 8192 columns per core
GLEN = 127 + T + 1     # 4224

_cache = {}


def _build_nc(reps=1, OB=2, XSPLIT=8, oeng="scalar", wbufs=8,
              pbufs=3, obufs=4, ceng="scalar", obf16=True, warmup=30,
              gend0=25, gbufs=3, gpbufs=2, geng="vector", nz=4):
    from concourse import bacc, tile
    import concourse.mybir as mybir

    NZ = nz
    SEEDY = C // NZ
    GROWS = C - (NZ - 1)
    NGEN = NB - gend0 if gend0 is not None else 0
    ND = NB - NGEN            # dense (HBM-streamed) offsets: d < ND
    nc = bacc.Bacc("TRN2", target_bir_lowering=False, debug=False)

    w_d = nc.dram_tensor("wdense", [CPC, C, ND * C], mybir.dt.bfloat16,
                         kind="ExternalInput")
    x_d = nc.dram_tensor("xmov", [C, COLS], mybir.dt.bfloat16,
                         kind="ExternalInput")
    if NGEN:
        seed_d = nc.dram_tensor("seeds", [CPC // 2, C, 2 * NGEN * SEEDY],
                                mybir.dt.bfloat16, kind="ExternalInput")
        strip_d = nc.dram_tensor("strips", [CPC, NZ - 1, NGEN * C],
                                 mybir.dt.bfloat16, kind="ExternalInput")
        shift_d = nc.dram_tensor("shifts", [C, NZ * C], mybir.dt.bfloat16,
                                 kind="ExternalInput")
    odt = mybir.dt.bfloat16 if obf16 else mybir.dt.float32
    o_d = nc.dram_tensor("out", [C, COLS], odt, kind="ExternalOutput")

    def _copy(eng, dst, src):
        if eng == "scalar":
            nc.scalar.activation(dst, src, mybir.ActivationFunctionType.Copy)
        else:
            getattr(nc, eng).tensor_copy(dst, src)

    with tile.TileContext(nc) as tc:
        with (
            tc.tile_pool(name="xpool", bufs=1) as xpool,
            tc.tile_pool(name="wpool", bufs=wbufs) as wpool,
            tc.tile_pool(name="opool", bufs=obufs) as opool,
            tc.tile_pool(name="spool", bufs=4) as spool,
            tc.tile_pool(name="gwpool", bufs=CPC) as gwpool,
            tc.tile_pool(name="psum", bufs=pbufs, space="PSUM") as psum,
            tc.tile_pool(name="gpsum", bufs=gpbufs, space="PSUM") as gpsum,
            tc.tile_pool(name="wupsum", bufs=1, space="PSUM") as wupsum,
        ):
            xmov = xpool.tile([C, COLS], mybir.dt.bfloat16)
            XCH = COLS // XSPLIT
            for k in range(XSPLIT):
                nc.scalar.dma_start(xmov[:, k * XCH:(k + 1) * XCH],
                                    x_d[:, k * XCH:(k + 1) * XCH])

            if NGEN:
                shifts = xpool.tile([C, NZ * C], mybir.dt.bfloat16,
                                    tag="shifts")
                nc.sync.dma_start(shifts[:], shift_d.ap())

            if warmup:
                # Keep the PE busy during the initial DMA window so the
                # HAM clock-gate reaches 8/8 before the first real matmul.
                wu = xpool.tile([C, C], mybir.dt.bfloat16, tag="warm")
                wups = wupsum.tile([C, C], mybir.dt.float32, tag="warmp")
                nc.vector.memset(wu[:], 0)
                for _ in range(warmup):
                    nc.tensor.matmul(wups[:], wu[:], wu[:], start=True,
                                     stop=True)

            # PSUM matmul outputs must stay inside one 2KB bank
            # (512 fp32 cols) -> generate in chunks of <=4 tiles.
            GCH = 4
            seed_tiles = {}
            def gen_tiles(j):
                cpeng = ["vector", "scalar"][j % 2] if geng == "alt" else geng
                """PE-generate channel j's tail weight tiles d>=gend0."""
                if j % 2 == 0:
                    sj = spool.tile([C, 2 * NGEN * SEEDY], mybir.dt.bfloat16,
                                    tag="seed")
                    nc.gpsimd.dma_start(sj[:], seed_d[j // 2])
                    seed_tiles[j + 1] = sj
                else:
                    sj = seed_tiles.pop(j)
                soff = (j % 2) * NGEN * SEEDY
                wg = gwpool.tile([C, NGEN * C], mybir.dt.bfloat16, tag="wg")
                nc.gpsimd.dma_start(wg[GROWS:C, :], strip_d[j])
                for c0 in range(0, NGEN, GCH):
                    c1 = min(c0 + GCH, NGEN)
                    nt = c1 - c0
                    pg = gpsum.tile([C, nt * C], mybir.dt.float32, tag="pg")
                    for z in range(NZ):
                        # out cols NZ*y+z of each chunk tile, one matmul
                        nc.tensor.matmul(
                            pg[:, z::NZ],
                            shifts[:, z * C:(z + 1) * C],
                            sj[:, soff + c0 * SEEDY:soff + c1 * SEEDY],
                            start=(z == 0),
                            stop=(z == NZ - 1),
                        )
                    _copy(cpeng, wg[0:GROWS, c0 * C:c1 * C], pg[0:GROWS, :])
                return wg

            LOOK = 2   # gen lookahead (channels) ahead of the main loop
            def body(_iv=None):
                wgs = {}
                if NGEN:
                    for j in range(LOOK):
                        wgs[j] = gen_tiles(j)
                for j in range(CPC):
                    if NGEN and j + LOOK < CPC:
                        wgs[j + LOOK] = gen_tiles(j + LOOK)
                    wj = wpool.tile([C, ND * C], mybir.dt.bfloat16)
                    nc.sync.dma_start(wj[:], w_d[j])
                    wg = wgs.pop(j, None)

                    acc = psum.tile([C, NB * B], mybir.dt.float32)
                    xj = xmov[:, j * NB * B:(j + 1) * NB * B]
                    for d in range(NB):
                        ncols = B * (NB - d)
                        wsrc = (wj[:, d * C:(d + 1) * C] if d < ND else
                                wg[:, (d - ND) * C:(d - ND + 1) * C])
                        nc.tensor.matmul(
                            acc[:, d * B:],
                            wsrc,
                            xj[:, :ncols],
                            start=(d == 0),
                            stop=(d == NB - 1),
                        )

                    if j % OB == 0:
                        og = opool.tile([C, OB * NB * B], odt, tag="og")
                    _copy(ceng,
                          og[:, (j % OB) * NB * B:(j % OB + 1) * NB * B],
                          acc[:])
                    if j % OB == OB - 1:
                        j0 = j - (OB - 1)
                        getattr(nc, oeng).dma_start(
                            o_d[:, j0 * NB * B:(j0 + OB) * NB * B], og[:])

            if reps == 1:
                body()
            else:
                with tc.For_i(0, reps, 1) as iv:
                    body(iv)

    nc.compile()
    return nc


def _prep_inputs(P, g, gend0=None, nz=None):
    """Host-side shard + layout prep. Returns in_maps list for 8 cores."""
    if gend0 is None:
        gend0 = KCFG.get("gend0")
    if nz is None:
        nz = KCFG.get("nz", 4)
    bf16 = ml_dtypes.bfloat16
    P = np.asarray(P)
    g = np.asarray(g)
    NZ = nz
    SEEDY = C // NZ
    NGEN = NB - gend0 if gend0 is not None else 0
    ND = NB - NGEN

    gmod = g.astype(np.float32).copy()
    gmod[0, :] += 1.0

    if NGEN:
        shifts = np.zeros((C, NZ * C), dtype=bf16)
        for z in range(NZ):
            shifts[:, z * C:(z + 1) * C] = np.eye(C, k=-z, dtype=np.float32)

    in_maps = []
    for core in range(N_CORES):
        lo, hi = core * CPC, (core + 1) * CPC
        gpads = np.zeros((CPC, GLEN), dtype=np.float32)
        gpads[:, 127:127 + T] = gmod[:, lo:hi].T
        gpads = gpads.astype(bf16)

        # Dense Toeplitz expansion for d < ND: wdense[j, p, e] = gpads[j, e+p]
        sw = np.lib.stride_tricks.sliding_window_view(gpads, ND * C, axis=1)
        wdense = np.ascontiguousarray(sw[:, :C, :])

        Pc = P[:, :, lo:hi]                                  # (B, T, CPC)
        x4 = Pc.reshape(B, NB, C, CPC)                       # (b, i, c, j)
        xmov = np.ascontiguousarray(
            x4[:, :, ::-1, :].transpose(2, 3, 1, 0)          # (p, j, i, b)
        ).reshape(C, COLS).astype(bf16)

        m = {"xmov": xmov, "wdense": wdense}
        if NGEN:
            # seeds[j, p, (d-ND)*SEEDY + y] = gpads[j, d*C + p + NZ*y]
            seeds = np.empty((CPC, C, NGEN * SEEDY), dtype=bf16)
            strips = np.empty((CPC, NZ - 1, NGEN * C), dtype=bf16)
            pidx = np.arange(C)[:, None]
            yidx = np.arange(SEEDY)[None, :]
            sidx = np.arange(NZ - 1)[:, None]
            cidx = np.arange(C)[None, :]
            for dd in range(NGEN):
                d = ND + dd
                seeds[:, :, dd * SEEDY:(dd + 1) * SEEDY] = \
                    gpads[:, d * C + pidx + NZ * yidx]
                strips[:, :, dd * C:(dd + 1) * C] = \
                    gpads[:, d * C + (C - (NZ - 1) + sidx) + cidx]
            # pair adjacent channels so each seed DMA moves >=512B rows
            m["seeds"] = np.ascontiguousarray(
                seeds.reshape(CPC // 2, 2, C, NGEN * SEEDY)
                .transpose(0, 2, 1, 3).reshape(CPC // 2, C, 2 * NGEN * SEEDY))
            m["strips"] = strips
            m["shifts"] = shifts
        in_maps.append(m)
    return in_maps


def _unshard(results):
    out = np.empty((B, T, NR), np.float32)
    for core in range(N_CORES):
        oc = np.asarray(results[core]["out"], dtype=np.float32)
        oc = oc.reshape(C, CPC, NB, B).transpose(3, 2, 0, 1)  # (b, i, a, j)
        out[:, :, core * CPC:(core + 1) * CPC] = oc.reshape(B, T, CPC)
    return out


KCFG = dict(OB=2, XSPLIT=8, wbufs=8, obf16=True, warmup=0, gend0=22, nz=4,
            gpbufs=4, geng="alt")


def kernel(P, g):
    from concourse.bass_utils import run_bass_kernel_spmd

    if "nc" not in _cache:
        _cache["nc"] = _build_nc(**KCFG)
    nc = _cache["nc"]

    in_maps = _prep_inputs(P, g, gend0=KCFG.get("gend0"),
                           nz=KCFG.get("nz", 4))
    res = run_bass_kernel_spmd(nc, in_maps, list(range(N_CORES)))
    return _unshard(res.results)


if __name__ == "__main__":
    rng = np.random.default_rng(0)
    P = rng.standard_normal((B, T, NR)).astype(np.float32)
    g = (rng.standard_normal((T, NR)) * 0.1).astype(np.float32)
    out = kernel(P, g)
    print("out shape:", out.shape, out.dtype)


# revision 19
# speedup vs baseline: 1.0372x; 1.0372x over previous
"""Trainium2 Bass kernel for nn_EpsiLayer: per-channel causal full-length
time convolution  out[b,t,j] = P[b,t,j] + sum_{k<=t} g[k,j] * P[b,t-k,j].

Identity fold: with g'[0] = g[0] + 1, out = causal_conv(g', P) exactly.

Per channel j the conv is a lower-triangular Toeplitz (T x T) matmul.
Blocked into C=128 chunks: y_i += W_d @ x_{i-d},
W_d[p, a] = gpad[d*128 + a + p], gpad = 127 zeros ++ g' (bf16); the
moving operand is time-reversed within each block on the host so the
contraction pairs line up.  Each W_d is a 128x128 Hankel slice of the
dense sliding window wdense[p, e] = gpad[e + p].

The kernel is HBM-bound on the weight stream (the dense Toeplitz
expansion is ~124x redundant but must be materialized in SBUF for the
PE's stationary operand; fp8/int8 fail the 2e-2 error budget).  Two
mitigations:
  - bf16 output (halves output traffic; host casts back to fp32).
  - PE-side Hankel generation for the tail offsets d >= GEND0: the
    host ships a compact seed S8_d[p, y] = gpad[d*128 + p + 8*y]
    ([128,16] bf16 = 4 KB vs 32 KB dense).  On device, 8 shift-matrix
    matmuls (lhsT = sub-diagonal permutation Sigma_z, z=0..7) scatter
    the seed into PSUM columns z::8, giving H_d[p, 8y+z] =
    S8_d[p+z, y] for p < 121; the last 7 partitions (p+z >= 128 would
    need seed rows past 128) come from a tiny dense HBM strip.  A DVE
    copy casts PSUM -> SBUF bf16.  This moves ~1/4 of the weight
    stream off HBM into spare PE cycles.

Sharding: channel-parallel, NR=256 -> 32 channels per core, all B and T
per core, zero inter-core communication.
"""

import sys
import numpy as np

try:
    from concourse import bacc, tile  # noqa: F401
except ImportError:  # grading env may not have it on sys.path yet
    sys.path.insert(0, "/opt/trn_rl_repo")

import ml_dtypes

B, T, NR = 8, 4096, 256
C = 128
NB = T // C            # 32 time blocks
N_CORES = 8
CPC = NR // N_CORES    # 32 channels per core
COLS = CPC * NB * B    # 8192 columns per core
GLEN = 127 + T + 1     # 4224

_cache = {}


def _build_nc(reps=1, OB=2, XSPLIT=8, oeng="scalar", wbufs=8,
              pbufs=3, obufs=4, ceng="scalar", obf16=True, warmup=30,
              gend0=25, gbufs=3, gpbufs=2, geng="vector", nz=4):
    from concourse import bacc, tile
    import concourse.mybir as mybir

    NZ = nz
    SEEDY = C // NZ
    GROWS = C - (NZ - 1)
    NGEN = NB - gend0 if gend0 is not None else 0
    ND = NB - NGEN            # dense (HBM-streamed) offsets: d < ND
    nc = bacc.Bacc("TRN2", target_bir_lowering=False, debug=False)

    w_d = nc.dram_tensor("wdense", [CPC, C, ND * C], mybir.dt.bfloat16,
                         kind="ExternalInput")
    x_d = nc.dram_tensor("xmov", [C, COLS], mybir.dt.bfloat16,
                         kind="ExternalInput")
    if NGEN:
        seed_d = nc.dram_tensor("seeds", [CPC // 2, C, 2 * NGEN * SEEDY],
                                mybir.dt.bfloat16, kind="ExternalInput")
        strip_d = nc.dram_tensor("strips", [CPC, NZ - 1, NGEN * C],
                                 mybir.dt.bfloat16, kind="ExternalInput")
        shift_d = nc.dram_tensor("shifts", [C, NZ * C], mybir.dt.bfloat16,
                                 kind="ExternalInput")
    odt = mybir.dt.bfloat16 if obf16 else mybir.dt.float32
    o_d = nc.dram_tensor("out", [C, COLS], odt, kind="ExternalOutput")

    def _copy(eng, dst, src):
        if eng == "scalar":
            nc.scalar.activation(dst, src, mybir.ActivationFunctionType.Copy)
        else:
            getattr(nc, eng).tensor_copy(dst, src)

    with tile.TileContext(nc) as tc:
        with (
            tc.tile_pool(name="xpool", bufs=1) as xpool,
            tc.tile_pool(name="wpool", bufs=wbufs) as wpool,
            tc.tile_pool(name="opool", bufs=obufs) as opool,
            tc.tile_pool(name="spool", bufs=4) as spool,
            tc.tile_pool(name="gwpool", bufs=CPC) as gwpool,
            tc.tile_pool(name="psum", bufs=pbufs, space="PSUM") as psum,
            tc.tile_pool(name="gpsum", bufs=gpbufs, space="PSUM") as gpsum,
            tc.tile_pool(name="wupsum", bufs=1, space="PSUM") as wupsum,
        ):
            xmov = xpool.tile([C, COLS], mybir.dt.bfloat16)
            XCH = COLS // XSPLIT
            for k in range(XSPLIT):
                nc.scalar.dma_start(xmov[:, k * XCH:(k + 1) * XCH],
                                    x_d[:, k * XCH:(k + 1) * XCH])

            if NGEN:
                shifts = xpool.tile([C, NZ * C], mybir.dt.bfloat16,
                                    tag="shifts")
                nc.sync.dma_start(shifts[:], shift_d.ap())

            if warmup:
                # Keep the PE busy during the initial DMA window so the
                # HAM clock-gate reaches 8/8 before the first real matmul.
                wu = xpool.tile([C, C], mybir.dt.bfloat16, tag="warm")
                wups = wupsum.tile([C, C], mybir.dt.float32, tag="warmp")
                nc.vector.memset(wu[:], 0)
                for _ in range(warmup):
                    nc.tensor.matmul(wups[:], wu[:], wu[:], start=True,
                                     stop=True)

            # PSUM matmul outputs must stay inside one 2KB bank
            # (512 fp32 cols) -> generate in chunks of <=4 tiles.
            GCH = 4
            seed_tiles = {}
            def gen_tiles(j):
                cpeng = ["vector", "scalar"][j % 2] if geng == "alt" else geng
                """PE-generate channel j's tail weight tiles d>=gend0."""
                if j % 2 == 0:
                    sj = spool.tile([C, 2 * NGEN * SEEDY], mybir.dt.bfloat16,
                                    tag="seed")
                    nc.gpsimd.dma_start(sj[:], seed_d[j // 2])
                    seed_tiles[j + 1] = sj
                else:
                    sj = seed_tiles.pop(j)
                soff = (j % 2) * NGEN * SEEDY
                wg = gwpool.tile([C, NGEN * C], mybir.dt.bfloat16, tag="wg")
                nc.gpsimd.dma_start(wg[GROWS:C, :], strip_d[j])
                for c0 in range(0, NGEN, GCH):
                    c1 = min(c0 + GCH, NGEN)
                    nt = c1 - c0
                    pg = gpsum.tile([C, nt * C], mybir.dt.float32, tag="pg")
                    for z in range(NZ):
                        # out cols NZ*y+z of each chunk tile, one matmul
                        nc.tensor.matmul(
                            pg[:, z::NZ],
                            shifts[:, z * C:(z + 1) * C],
                            sj[:, soff + c0 * SEEDY:soff + c1 * SEEDY],
                            start=(z == 0),
                            stop=(z == NZ - 1),
                        )
                    _copy(cpeng, wg[0:GROWS, c0 * C:c1 * C], pg[0:GROWS, :])
                return wg

            LOOK = 2   # gen lookahead (channels) ahead of the main loop
            def body(_iv=None):
                wgs = {}
                if NGEN:
                    for j in range(LOOK):
                        wgs[j] = gen_tiles(j)
                for j in range(CPC):
                    if NGEN and j + LOOK < CPC:
                        wgs[j + LOOK] = gen_tiles(j + LOOK)
                    wj = wpool.tile([C, ND * C], mybir.dt.bfloat16)
                    nc.sync.dma_start(wj[:], w_d[j])
                    wg = wgs.pop(j, None)

                    acc = psum.tile([C, NB * B], mybir.dt.float32)
                    xj = xmov[:, j * NB * B:(j + 1) * NB * B]
                    for d in range(NB):
                        ncols = B * (NB - d)
                        wsrc = (wj[:, d * C:(d + 1) * C] if d < ND else
                                wg[:, (d - ND) * C:(d - ND + 1) * C])
                        nc.tensor.matmul(
                            acc[:, d * B:],
                            wsrc,
                            xj[:, :ncols],
                            start=(d == 0),
                            stop=(d == NB - 1),
                        )

                    if j % OB == 0:
                        og = opool.tile([C, OB * NB * B], odt, tag="og")
                    _copy(ceng,
                          og[:, (j % OB) * NB * B:(j % OB + 1) * NB * B],
                          acc[:])
                    if j % OB == OB - 1:
                        j0 = j - (OB - 1)
                        getattr(nc, oeng).dma_start(
                            o_d[:, j0 * NB * B:(j0 + OB) * NB * B], og[:])

            if reps == 1:
                body()
            else:
                with tc.For_i(0, reps, 1) as iv:
                    body(iv)

    nc.compile()
    return nc


def _prep_inputs(P, g, gend0=None, nz=None):
    """Host-side shard + layout prep. Returns in_maps list for 8 cores."""
    if gend0 is None:
        gend0 = KCFG.get("gend0")
    if nz is None:
        nz = KCFG.get("nz", 4)
    bf16 = ml_dtypes.bfloat16
    P = np.asarray(P)
    g = np.asarray(g)
    NZ = nz
    SEEDY = C // NZ
    NGEN = NB - gend0 if gend0 is not None else 0
    ND = NB - NGEN

    gmod = g.astype(np.float32).copy()
    gmod[0, :] += 1.0

    if NGEN:
        shifts = np.zeros((C, NZ * C), dtype=bf16)
        for z in range(NZ):
            shifts[:, z * C:(z + 1) * C] = np.eye(C, k=-z, dtype=np.float32)

    in_maps = []
    for core in range(N_CORES):
        lo, hi = core * CPC, (core + 1) * CPC
        gpads = np.zeros((CPC, GLEN), dtype=np.float32)
        gpads[:, 127:127 + T] = gmod[:, lo:hi].T
        gpads = gpads.astype(bf16)

        # Dense Toeplitz expansion for d < ND: wdense[j, p, e] = gpads[j, e+p]
        sw = np.lib.stride_tricks.sliding_window_view(gpads, ND * C, axis=1)
        wdense = np.ascontiguousarray(sw[:, :C, :])

        Pc = P[:, :, lo:hi]                                  # (B, T, CPC)
        x4 = Pc.reshape(B, NB, C, CPC)                       # (b, i, c, j)
        xmov = np.ascontiguousarray(
            x4[:, :, ::-1, :].transpose(2, 3, 1, 0)          # (p, j, i, b)
        ).reshape(C, COLS).astype(bf16)

        m = {"xmov": xmov, "wdense": wdense}
        if NGEN:
            # seeds[j, p, (d-ND)*SEEDY + y] = gpads[j, d*C + p + NZ*y]
            seeds = np.empty((CPC, C, NGEN * SEEDY), dtype=bf16)
            strips = np.empty((CPC, NZ - 1, NGEN * C), dtype=bf16)
            pidx = np.arange(C)[:, None]
            yidx = np.arange(SEEDY)[None, :]
            sidx = np.arange(NZ - 1)[:, None]
            cidx = np.arange(C)[None, :]
            for dd in range(NGEN):
                d = ND + dd
                seeds[:, :, dd * SEEDY:(dd + 1) * SEEDY] = \
                    gpads[:, d * C + pidx + NZ * yidx]
                strips[:, :, dd * C:(dd + 1) * C] = \
                    gpads[:, d * C + (C - (NZ - 1) + sidx) + cidx]
            # pair adjacent channels so each seed DMA moves >=512B rows
            m["seeds"] = np.ascontiguousarray(
                seeds.reshape(CPC // 2, 2, C, NGEN * SEEDY)
                .transpose(0, 2, 1, 3).reshape(CPC // 2, C, 2 * NGEN * SEEDY))
            m["strips"] = strips
            m["shifts"] = shifts
        in_maps.append(m)
    return in_maps


def _unshard(results):
    out = np.empty((B, T, NR), np.float32)
    for core in range(N_CORES):
        oc = np.asarray(results[core]["out"], dtype=np.float32)
        oc = oc.reshape(C, CPC, NB, B).transpose(3, 2, 0, 1)  # (b, i, a, j)
        out[:, :, core * CPC:(core + 1) * CPC] = oc.reshape(B, T, CPC)
    return out


KCFG = dict(OB=2, XSPLIT=8, wbufs=8, obf16=True, warmup=0, gend0=24, nz=4,
            gpbufs=4, geng="alt")


def kernel(P, g):
    from concourse.bass_utils import run_bass_kernel_spmd

    if "nc" not in _cache:
        _cache["nc"] = _build_nc(**KCFG)
    nc = _cache["nc"]

    in_maps = _prep_inputs(P, g, gend0=KCFG.get("gend0"),
                           nz=KCFG.get("nz", 4))
    res = run_bass_kernel_spmd(nc, in_maps, list(range(N_CORES)))
    return _unshard(res.results)


if __name__ == "__main__":
    rng = np.random.default_rng(0)
    P = rng.standard_normal((B, T, NR)).astype(np.float32)
    g = (rng.standard_normal((T, NR)) * 0.1).astype(np.float32)
    out = kernel(P, g)
    print("out shape:", out.shape, out.dtype)
